# revision 19
# baseline (speedup 1.0000x reference)
"""Trainium2 Bass kernel for ExpanderLinearLayer (gather-mul-scatter_add).

Reformulation: out = input_ @ S + bias, where S[i, j] = sum of weight[k] over
all k with ind_in[k] == i and ind_out[k] == j.  S is built dense on the host
(52224 nnz into 1024x1024) and the device runs a dense bf16 matmul,
data-parallel over the batch across 8 NeuronCores.

Per core (batch shard of 512 rows): the 1024-long contraction dim is split
into 8 chunks of 128 partitions.  All device I/O is bf16 (errors ~2e-3 vs
the 2e-2 gate): input DMA is 3.0 MB instead of 6.3, output 1 MB instead of 2.

  chunk 0 is split into two DMAs ([x_0 | s_0 m0,m1] and [s_0 m2..7]) so the
  first matmul can start ~0.5us after data starts flowing.
  chunk k (k>=1):  [x_k | s_k]  at cols [k*1536, (k+1)*1536)
      x_k[p, n] = input_[c*512+n, k*128+p]   (n < 512)
      s_k[p, m*128+q] = S[k*128+p, m*128+q]
  bias arrives as a separate tiny f32 tensor [128, 8] on the scalar ring.

Matmul (k outer, m inner): psum[m] += s_km.T @ x_k, accumulated over k in 8
PSUM banks.  A few junk "prewarm" matmuls run during the DMA fill so the PE
HAM clock-gate (1.2 GHz cold -> 2.4 GHz warm after ~3.4us of activity) is
released before the real stream starts.

Epilogue streams: as each psum bank finishes (k=7), bias-add + bf16 cast on
Vector (even m) / Scalar-ACT Identity (odd m), then paired output DMAs on
the two HWDGE rings so the tail after the last matmul is short.
"""

import os
import numpy as np
import ml_dtypes

try:
    from concourse import bacc, bass, mybir
    from concourse.tile import TileContext
    from concourse.bass_utils import run_bass_kernel_spmd
except ImportError:  # fresh dir without PYTHONPATH
    import sys

    sys.path.insert(0, "/opt/trn_rl_repo")
    from concourse import bacc, bass, mybir
    from concourse.tile import TileContext
    from concourse.bass_utils import run_bass_kernel_spmd

P = 128
B = 4096
D = 1024
NCORES = 8
BS = B // NCORES      # 512 batch rows per core
KO = D // P           # 8 contraction chunks
MO = D // P           # 8 output tiles
CW = BS + D           # 1536 columns per merged chunk
NWARM = 23            # junk matmuls to pre-warm the PE HAM clock gate
KSPLIT = 4            # chunks 0..KSPLIT-1 run k-outer; rest run bank-major

F32 = mybir.dt.float32
BF16 = mybir.dt.bfloat16
BF16_NP = ml_dtypes.bfloat16

_NC_CACHE = {}
LAST_RESULTS = None


def _build_nc():
    nc = bacc.Bacc("TRN2", target_bir_lowering=False)
    xs_d = nc.declare_dram_parameter("xs", [P, KO * CW], BF16, isOutput=False)
    bs_d = nc.declare_dram_parameter("bs", [P, MO], F32, isOutput=False)
    o_d = nc.declare_dram_parameter("o", [P, MO, BS], BF16, isOutput=True)

    with TileContext(nc) as tc:
        with (
            tc.tile_pool(name="cs", bufs=1) as cpool,
            tc.tile_pool(name="bb", bufs=1) as bpool,
            tc.tile_pool(name="ob", bufs=1) as opool,
            tc.tile_pool(name="wj", bufs=1) as wpool,
            tc.tile_pool(name="ps", bufs=1, space="PSUM") as pspool,
        ):
            psums = [
                pspool.tile([P, BS], F32, tag=f"ps{m}", name=f"ps{m}")
                for m in range(MO)
            ]

            # PE prewarm: junk matmuls with no DMA dependency keep the PE
            # busy during the DMA fill so HAM releases the clock gate
            # before the real stream starts.  N=128 keeps the granularity
            # fine so the real stream isn't delayed when chunk 0 lands.
            # The memset lives on Vector (idle in the prologue) so the
            # first warmup can start right after the barrier.
            junk = wpool.tile([P, P], BF16, tag="junk", name="junk")
            nc.vector.memset(junk, 0.0)
            for w in range(NWARM):
                nc.tensor.matmul(
                    psums[0][:, :P], lhsT=junk, rhs=junk, start=True, stop=True
                )

            bias_sb = bpool.tile([P, MO], F32, tag="bias", name="bias")
            nc.scalar.dma_start(bias_sb, bs_d[:, :])

            # Input chunks on the sync HWDGE ring, in consumption order.
            # Chunks 0 and 1 are split in two DMAs each so their first
            # matmuls aren't gated on the whole chunk (the k=0/k=1 phases
            # run closest behind the DMA stream); chunks 2-7 arrive with
            # plenty of slack as single DMAs.  Do NOT merge chunks: a
            # merged DMA's semaphore only fires after the LAST chunk's
            # data, stalling the earlier chunk's matmuls ~2us.
            split = {0, 1}
            parts = {}
            for k in range(KO):
                if k in split:
                    ca = cpool.tile([P, BS + 2 * P], BF16, tag=f"c{k}a", name=f"c{k}a")
                    cb = cpool.tile([P, 6 * P], BF16, tag=f"c{k}b", name=f"c{k}b")
                    nc.sync.dma_start(ca, xs_d[:, k * CW : k * CW + BS + 2 * P])
                    nc.sync.dma_start(cb, xs_d[:, k * CW + BS + 2 * P : (k + 1) * CW])
                    parts[k] = (ca, cb)
                else:
                    ct = cpool.tile([P, CW], BF16, tag=f"c{k}", name=f"c{k}")
                    nc.sync.dma_start(ct, xs_d[:, k * CW : (k + 1) * CW])
                    parts[k] = (ct,)

            def chunk_x(k):
                return parts[k][0][:, :BS]

            def chunk_s(k, m):
                if k in split:
                    ca, cb = parts[k]
                    if m < 2:
                        return ca[:, BS + m * P : BS + (m + 1) * P]
                    return cb[:, (m - 2) * P : (m - 1) * P]
                return parts[k][0][:, BS + m * P : BS + (m + 1) * P]

            # Phase A (k-outer, m-inner): chunks 0..KSPLIT-1 consumed in
            # DMA-arrival order while later chunks stream in.
            for k in range(KSPLIT):
                rhs = chunk_x(k)
                for m in range(MO):
                    nc.tensor.matmul(
                        psums[m],
                        lhsT=chunk_s(k, m),
                        rhs=rhs,
                        start=(k == 0),
                        stop=False,
                    )

            # Phase B (bank-major): by the time the PE gets here, all
            # chunks are (nearly) resident, so finish one psum bank at a
            # time.  Bank m completes ~0.9us apart, letting the bias-add
            # and its output DMA stream underneath the matmuls instead of
            # piling up after the last one.
            out_sb = opool.tile([P, MO, BS], BF16, tag="out", name="out")
            for m in range(MO):
                for k in range(KSPLIT, KO):
                    nc.tensor.matmul(
                        psums[m],
                        lhsT=chunk_s(k, m),
                        rhs=chunk_x(k),
                        start=False,
                        stop=(k == KO - 1),
                    )
                if m == MO - 1:
                    # Last bank is the kernel's critical tail.  Half-sized
                    # ACT+DMA pairs on the scalar engine/ring let the first
                    # half's data drain while the second half computes.
                    H = BS // 2
                    for h in range(2):
                        sl = slice(h * H, (h + 1) * H)
                        nc.scalar.activation(
                            out_sb[:, m, sl],
                            psums[m][:, sl],
                            mybir.ActivationFunctionType.Identity,
                            bias=bias_sb[:, m : m + 1],
                        )
                        nc.scalar.dma_start(o_d[:, m, sl], out_sb[:, m, sl])
                elif m % 2 == 0:
                    nc.vector.tensor_scalar_add(
                        out_sb[:, m], psums[m], bias_sb[:, m : m + 1]
                    )
                    nc.sync.dma_start(o_d[:, m], out_sb[:, m])
                else:
                    nc.scalar.activation(
                        out_sb[:, m],
                        psums[m],
                        mybir.ActivationFunctionType.Identity,
                        bias=bias_sb[:, m : m + 1],
                    )
                    nc.sync.dma_start(o_d[:, m], out_sb[:, m])

    nc.finalize()
    return nc


def _get_nc():
    if "nc" not in _NC_CACHE:
        _NC_CACHE["nc"] = _build_nc()
    return _NC_CACHE["nc"]


def kernel(input_, weight, bias, ind_in, ind_out):
    global LAST_RESULTS
    input_ = np.asarray(input_, dtype=np.float32)
    weight = np.asarray(weight, dtype=np.float32)
    bias = np.asarray(bias, dtype=np.float32)
    ind_in = np.asarray(ind_in, dtype=np.int64)
    ind_out = np.asarray(ind_out, dtype=np.int64)

    # Dense scatter matrix S (f32 accumulate, then bf16).
    S = np.zeros((D, D), np.float32)
    np.add.at(S, (ind_in, ind_out), weight)
    Sb = S.astype(BF16_NP)
    b_l = np.ascontiguousarray(bias.reshape(MO, P).T)  # [128, 8] f32

    in_maps = []
    for c in range(NCORES):
        xT = input_[c * BS : (c + 1) * BS].T.astype(BF16_NP)  # [1024, 512]
        xs_l = np.empty((P, KO * CW), BF16_NP)
        for k in range(KO):
            rows = slice(k * P, (k + 1) * P)
            off = k * CW
            xs_l[:, off : off + BS] = xT[rows]
            xs_l[:, off + BS : off + CW] = Sb[rows]
        in_maps.append({"xs": xs_l, "bs": b_l})

    nc = _get_nc()
    res = run_bass_kernel_spmd(
        nc,
        in_maps,
        core_ids=list(range(NCORES)),
        trace=bool(int(os.environ.get("KERNEL_TRACE", "0"))),
    )
    LAST_RESULTS = res

    outs = []
    for c in range(NCORES):
        o = np.asarray(res.results[c]["o"])
        outT = (
            o.reshape(P, MO, BS).transpose(1, 0, 2).reshape(D, BS).astype(np.float32)
        )
        outs.append(outT.T)
    return np.ascontiguousarray(np.concatenate(outs, axis=0))


# revision 22
# speedup vs baseline: 1.0479x; 1.0479x over previous
"""Trainium2 Bass kernel for ExpanderLinearLayer (gather-mul-scatter_add).

Reformulation: out = input_ @ S + bias, where S[i, j] = sum of weight[k] over
all k with ind_in[k] == i and ind_out[k] == j.  S is built dense on the host
(52224 nnz into 1024x1024) and the device runs a dense bf16 matmul,
data-parallel over the batch across 8 NeuronCores.

Per core (batch shard of 512 rows): the 1024-long contraction dim is split
into 8 chunks of 128 partitions.  All device I/O is bf16 (errors ~2e-3 vs
the 2e-2 gate): input DMA is 3.0 MB instead of 6.3, output 1 MB instead of 2.

  chunk 0 is split into two DMAs ([x_0 | s_0 m0,m1] and [s_0 m2..7]) so the
  first matmul can start ~0.5us after data starts flowing.
  chunk k (k>=1):  [x_k | s_k]  at cols [k*1536, (k+1)*1536)
      x_k[p, n] = input_[c*512+n, k*128+p]   (n < 512)
      s_k[p, m*128+q] = S[k*128+p, m*128+q]
  bias arrives as a separate tiny f32 tensor [128, 8] on the scalar ring.

Matmul (k outer, m inner): psum[m] += s_km.T @ x_k, accumulated over k in 8
PSUM banks.  A few junk "prewarm" matmuls run during the DMA fill so the PE
HAM clock-gate (1.2 GHz cold -> 2.4 GHz warm after ~3.4us of activity) is
released before the real stream starts.

Epilogue streams: as each psum bank finishes (k=7), bias-add + bf16 cast on
Vector (even m) / Scalar-ACT Identity (odd m), then paired output DMAs on
the two HWDGE rings so the tail after the last matmul is short.
"""

import os
import numpy as np
import ml_dtypes

try:
    from concourse import bacc, bass, mybir
    from concourse.tile import TileContext
    from concourse.bass_utils import run_bass_kernel_spmd
except ImportError:  # fresh dir without PYTHONPATH
    import sys

    sys.path.insert(0, "/opt/trn_rl_repo")
    from concourse import bacc, bass, mybir
    from concourse.tile import TileContext
    from concourse.bass_utils import run_bass_kernel_spmd

P = 128
B = 4096
D = 1024
NCORES = 8
BS = B // NCORES      # 512 batch rows per core
KO = D // P           # 8 contraction chunks
MO = D // P           # 8 output tiles
CW = BS + D           # 1536 columns per merged chunk
NWARM = 23            # junk matmuls to pre-warm the PE HAM clock gate
KSPLIT = 5            # chunks 0..KSPLIT-1 run k-outer; rest run bank-major

F32 = mybir.dt.float32
BF16 = mybir.dt.bfloat16
BF16_NP = ml_dtypes.bfloat16

_NC_CACHE = {}
LAST_RESULTS = None


def _build_nc():
    nc = bacc.Bacc("TRN2", target_bir_lowering=False)
    xs_d = nc.declare_dram_parameter("xs", [P, KO * CW], BF16, isOutput=False)
    bs_d = nc.declare_dram_parameter("bs", [P, MO], F32, isOutput=False)
    o_d = nc.declare_dram_parameter("o", [P, MO, BS], BF16, isOutput=True)

    with TileContext(nc) as tc:
        with (
            tc.tile_pool(name="cs", bufs=1) as cpool,
            tc.tile_pool(name="bb", bufs=1) as bpool,
            tc.tile_pool(name="ob", bufs=1) as opool,
            tc.tile_pool(name="wj", bufs=1) as wpool,
            tc.tile_pool(name="ps", bufs=1, space="PSUM") as pspool,
        ):
            psums = [
                pspool.tile([P, BS], F32, tag=f"ps{m}", name=f"ps{m}")
                for m in range(MO)
            ]

            # PE prewarm: junk matmuls with no DMA dependency keep the PE
            # busy during the DMA fill so HAM releases the clock gate
            # before the real stream starts.  N=128 keeps the granularity
            # fine so the real stream isn't delayed when chunk 0 lands.
            # The memset lives on Vector (idle in the prologue) so the
            # first warmup can start right after the barrier.
            junk = wpool.tile([P, P], BF16, tag="junk", name="junk")
            nc.vector.memset(junk, 0.0)
            for w in range(NWARM):
                nc.tensor.matmul(
                    psums[0][:, :P], lhsT=junk, rhs=junk, start=True, stop=True
                )

            bias_sb = bpool.tile([P, MO], F32, tag="bias", name="bias")
            nc.scalar.dma_start(bias_sb, bs_d[:, :])

            # Input chunks on the sync HWDGE ring, in consumption order.
            # Chunks 0 and 1 are split in two DMAs each so their first
            # matmuls aren't gated on the whole chunk (the k=0/k=1 phases
            # run closest behind the DMA stream); chunks 2-7 arrive with
            # plenty of slack as single DMAs.  Do NOT merge chunks: a
            # merged DMA's semaphore only fires after the LAST chunk's
            # data, stalling the earlier chunk's matmuls ~2us.
            split = {0, 1}
            parts = {}
            for k in range(KO):
                if k in split:
                    ca = cpool.tile([P, BS + 2 * P], BF16, tag=f"c{k}a", name=f"c{k}a")
                    cb = cpool.tile([P, 6 * P], BF16, tag=f"c{k}b", name=f"c{k}b")
                    nc.sync.dma_start(ca, xs_d[:, k * CW : k * CW + BS + 2 * P])
                    nc.sync.dma_start(cb, xs_d[:, k * CW + BS + 2 * P : (k + 1) * CW])
                    parts[k] = (ca, cb)
                else:
                    ct = cpool.tile([P, CW], BF16, tag=f"c{k}", name=f"c{k}")
                    nc.sync.dma_start(ct, xs_d[:, k * CW : (k + 1) * CW])
                    parts[k] = (ct,)

            def chunk_x(k):
                return parts[k][0][:, :BS]

            def chunk_s(k, m):
                if k in split:
                    ca, cb = parts[k]
                    if m < 2:
                        return ca[:, BS + m * P : BS + (m + 1) * P]
                    return cb[:, (m - 2) * P : (m - 1) * P]
                return parts[k][0][:, BS + m * P : BS + (m + 1) * P]

            # Phase A (k-outer, m-inner): chunks 0..KSPLIT-1 consumed in
            # DMA-arrival order while later chunks stream in.
            for k in range(KSPLIT):
                rhs = chunk_x(k)
                for m in range(MO):
                    nc.tensor.matmul(
                        psums[m],
                        lhsT=chunk_s(k, m),
                        rhs=rhs,
                        start=(k == 0),
                        stop=False,
                    )

            # Phase B (bank-major): by the time the PE gets here, all
            # chunks are (nearly) resident, so finish one psum bank at a
            # time.  Bank m completes ~0.9us apart, letting the bias-add
            # and its output DMA stream underneath the matmuls instead of
            # piling up after the last one.
            out_sb = opool.tile([P, MO, BS], BF16, tag="out", name="out")
            for m in range(MO):
                for k in range(KSPLIT, KO):
                    nc.tensor.matmul(
                        psums[m],
                        lhsT=chunk_s(k, m),
                        rhs=chunk_x(k),
                        start=False,
                        stop=(k == KO - 1),
                    )
                if m % 2 == 0:
                    nc.vector.tensor_scalar_add(
                        out_sb[:, m], psums[m], bias_sb[:, m : m + 1]
                    )
                    nc.sync.dma_start(o_d[:, m], out_sb[:, m])
                else:
                    nc.scalar.activation(
                        out_sb[:, m],
                        psums[m],
                        mybir.ActivationFunctionType.Identity,
                        bias=bias_sb[:, m : m + 1],
                    )
                    # Last bank's DMA rides the scalar ring (same-engine
                    # FIFO after its ACT); sync may still be issuing m6's.
                    eng = nc.scalar if m == MO - 1 else nc.sync
                    eng.dma_start(o_d[:, m], out_sb[:, m])

    nc.finalize()
    return nc


def _get_nc():
    if "nc" not in _NC_CACHE:
        _NC_CACHE["nc"] = _build_nc()
    return _NC_CACHE["nc"]


def kernel(input_, weight, bias, ind_in, ind_out):
    global LAST_RESULTS
    input_ = np.asarray(input_, dtype=np.float32)
    weight = np.asarray(weight, dtype=np.float32)
    bias = np.asarray(bias, dtype=np.float32)
    ind_in = np.asarray(ind_in, dtype=np.int64)
    ind_out = np.asarray(ind_out, dtype=np.int64)

    # Dense scatter matrix S (f32 accumulate, then bf16).
    S = np.zeros((D, D), np.float32)
    np.add.at(S, (ind_in, ind_out), weight)
    Sb = S.astype(BF16_NP)
    b_l = np.ascontiguousarray(bias.reshape(MO, P).T)  # [128, 8] f32

    in_maps = []
    for c in range(NCORES):
        xT = input_[c * BS : (c + 1) * BS].T.astype(BF16_NP)  # [1024, 512]
        xs_l = np.empty((P, KO * CW), BF16_NP)
        for k in range(KO):
            rows = slice(k * P, (k + 1) * P)
            off = k * CW
            xs_l[:, off : off + BS] = xT[rows]
            xs_l[:, off + BS : off + CW] = Sb[rows]
        in_maps.append({"xs": xs_l, "bs": b_l})

    nc = _get_nc()
    res = run_bass_kernel_spmd(
        nc,
        in_maps,
        core_ids=list(range(NCORES)),
        trace=bool(int(os.environ.get("KERNEL_TRACE", "0"))),
    )
    LAST_RESULTS = res

    outs = []
    for c in range(NCORES):
        o = np.asarray(res.results[c]["o"])
        outT = (
            o.reshape(P, MO, BS).transpose(1, 0, 2).reshape(D, BS).astype(np.float32)
        )
        outs.append(outT.T)
    return np.ascontiguousarray(np.concatenate(outs, axis=0))
